# revision 26
# baseline (speedup 1.0000x reference)
"""CondConv (per-sample routed 3x3 conv) on 8 Trainium2 NeuronCores.

Reference computation (all fp32):
    gap     = mean(x, axis=(2,3))                    [B, CIN]
    routing = sigmoid(gap @ W_att.T + b_att)         [B, E]
    ker     = einsum('be,eoihw->boihw', routing, convs)
    out[b]  = conv2d(x[b], ker[b], stride 1, pad 1)  [B, COUT, 56, 56]

Sharding (B=32, COUT=256 across 8 cores): 4 core-pairs; pair p owns
samples 8p..8p+7 (batch data-parallel), and within a pair each core
computes one half of COUT (128 channels).

Pipeline dtype is bf16 (tolerance 2e-2 >> bf16 conv error ~5e-3).
PSUM accumulation stays fp32; outputs drain as fp32.

Per-core program (SPMD — same program, different data). The conv
matmul stream is the roofline (1008 MMs x 448 cols @ 2.4 GHz = 188us);
everything else is engineered to hide under it:
  - expert bank cv[e] [128cin, 2c*9s*128m] resident in SBUF (bf16)
  - GAP via ScalarE Copy+accum halves, overlapping the input DMA
  - routing on DVE/GPSIMD/ScalarE (PE queue stays pure conv)
  - kernel mix as a product tree: 8 tensor_scalar products (4 on
    ScalarE via activation-scale, 4 on VectorE at 2x bf16 packing)
    folded by 6 VectorE + 1 GpSimd tensor_adds. ~14us spread over
    three engines per 23.5us conv window (the old single-engine
    scalar_tensor_tensor chain ran 1x and saturated VectorE).
  - conv in two tile-groups (rows 0-31, 32-55) so PSUM drains spread
    mid-sample and the next sample's first matmuls never wait on a
    bank; drains alternate VectorE/ScalarE; last sample drains in
    half-tiles on both engines to shrink the tail.
  - sample 0's mix is emitted in 384-col splits per chunk so the
    first conv matmul issues right after the ~13us input-DMA gate.
"""

import numpy as np

B, CIN, H, W = 32, 256, 56, 56
COUT, KK, E = 256, 3, 8
HP, WP = H + 2, W + 2          # zero-padded input plane
PHW = HP * WP                  # 3364
NSH = KK * KK                  # 9 shifts
CHUNKS = 2                     # CIN = 2 * 128
MHALF = COUT // 2              # couts per core
ROWS_PER_TILE = 8              # output rows per matmul tile
NTILES = H // ROWS_PER_TILE    # 7
NFREE = ROWS_PER_TILE * W      # 448
NCORES = 8
SAMPLES_PER_CORE = B // (NCORES // 2)  # 8
KCOLS = CHUNKS * NSH * 128     # kt / bank tile columns (2304)

ACT_EXPERTS = (0, 1, 2)        # products on ScalarE (activation scale)
TILE_GROUPS = ((0, 4), (4, 7))

_cached = {}


def _build_program():
    import concourse.bacc as bacc
    import concourse.bass_isa as bass_isa
    import concourse.mybir as mybir
    from concourse.tile import TileContext

    f32 = mybir.dt.float32
    bf16 = mybir.dt.bfloat16
    Alu = mybir.AluOpType
    Act = mybir.ActivationFunctionType

    nc = bacc.Bacc(None, target_bir_lowering=False)

    xpad_d = nc.declare_dram_parameter(
        "xpad", [SAMPLES_PER_CORE, CHUNKS, 128, PHW], bf16, isOutput=False)
    # chunk-major so the whole bank loads in 2 DMA issues (dma_start costs
    # ~0.6us of Sync-engine issue time each; 16 issues would gate startup)
    convsT_d = nc.declare_dram_parameter(
        "convsT", [CHUNKS, 128, E * NSH * 128], bf16, isOutput=False)
    watt_d = nc.declare_dram_parameter("watt", [CHUNKS, 128, E], f32, isOutput=False)
    battb_d = nc.declare_dram_parameter("battb", [128, E], f32, isOutput=False)
    out_d = nc.declare_dram_parameter(
        "out", [SAMPLES_PER_CORE, MHALF, H, W], f32, isOutput=True)

    with TileContext(nc) as tc:
        with (
            tc.tile_pool(name="resident", bufs=1) as res_pool,
            tc.tile_pool(name="xp", bufs=3) as xp_pool,
            tc.tile_pool(name="kt", bufs=3) as kt_pool,
            tc.tile_pool(name="prod", bufs=1) as prod_pool,
            tc.tile_pool(name="small", bufs=3) as small_pool,
            tc.tile_pool(name="outsb", bufs=6) as out_pool,
            tc.tile_pool(name="cpsum", bufs=1, space="PSUM") as cps_pool,
        ):
            # ---- small resident tiles -------------------------------------
            watt_sb = []
            for c in range(CHUNKS):
                t = res_pool.tile([128, E], f32, name=f"watt{c}", tag=f"watt{c}")
                nc.sync.dma_start(out=t[:], in_=watt_d[c])
                watt_sb.append(t)
            battb_sb = res_pool.tile([128, E], f32, name="battb", tag="battb")
            nc.sync.dma_start(out=battb_sb[:], in_=battb_d[:])
            # broadcast routing weights: scal[:, 8*b+e] = r_be on every partition
            scal_sb = res_pool.tile([128, SAMPLES_PER_CORE * E], f32,
                                    name="scal", tag="scal")

            # expert bank: one resident tile, cols e*2304 + c*1152 + s*128 + m
            bank_sb = res_pool.tile([128, E * KCOLS], bf16, name="bank",
                                    tag="bank")
            cv_sb = [bank_sb[:, e * KCOLS:(e + 1) * KCOLS] for e in range(E)]
            # mix product scratch
            prod = [prod_pool.tile([128, KCOLS], bf16, name=f"p{e}",
                                   tag=f"p{e}") for e in range(E)]

            def emit_bank_chunk(c, splits=None):
                """Load bank chunk c, optionally in column-splits. A single
                dma_start only signals completion as a whole; column-splits
                let prologue mix ranges start as their columns land."""
                hc_ = NSH * 128
                view = bank_sb[:].rearrange("p (e k) -> p e k", k=KCOLS)
                iview = convsT_d[c].rearrange("p (e m) -> p e m", m=hc_)
                for lo, hi in (splits or [(0, hc_)]):
                    nc.sync.dma_start(
                        out=view[:, :, c * hc_ + lo:c * hc_ + hi],
                        in_=iview[:, :, lo:hi])

            def emit_load_dma(b):
                """DMA padded input for sample b: one issue per chunk.
                dma_start costs ~0.6us of Sync issue time and startup
                bandwidth is issue-rate-bound, so fewer+bigger wins."""
                xp = []
                for c in range(CHUNKS):
                    t = xp_pool.tile([128, PHW], bf16, name=f"xp{c}", tag=f"xp{c}")
                    nc.sync.dma_start(out=t[:], in_=xpad_d[b, c])
                    xp.append(t)
                return xp

            def emit_load_gap(xp):
                """GAP pass: ScalarE in-place Copy whose accum_out yields the
                per-chunk row sums."""
                gq = []
                for c in range(CHUNKS):
                    g = small_pool.tile([128, 1], f32, name=f"gh{c}",
                                        tag=f"gh{c}")
                    nc.scalar.activation(out=xp[c][:], in_=xp[c][:],
                                         func=Act.Copy, accum_out=g[:])
                    gq.append(g)
                return gq

            def emit_load(b):
                xp = emit_load_dma(b)
                return xp, emit_load_gap(xp)

            def emit_routing(b, gs):
                """Routing for sample b on DVE/GPSIMD/ScalarE only.

                logits[e] = sum_cin gap[cin] * W_att[e,cin] / 3136 + b_att[e]
                (the 1/3136 is folded into watt host-side).
                """
                gsum = gs
                t0 = small_pool.tile([128, E], f32, name="t0", tag="t0")
                nc.vector.tensor_scalar_mul(out=t0[:], in0=watt_sb[0][:],
                                            scalar1=gsum[0][:, 0:1])
                t1 = small_pool.tile([128, E], f32, name="t1", tag="t1")
                nc.vector.scalar_tensor_tensor(
                    out=t1[:], in0=watt_sb[1][:], scalar=gsum[1][:, 0:1],
                    in1=t0[:], op0=Alu.mult, op1=Alu.add)
                red = small_pool.tile([128, E], f32, name="red", tag="red")
                nc.gpsimd.partition_all_reduce(red[:], t1[:], channels=128,
                                               reduce_op=bass_isa.ReduceOp.add)
                red2 = small_pool.tile([128, E], f32, name="red2", tag="red2")
                nc.vector.tensor_add(out=red2[:], in0=red[:], in1=battb_sb[:])
                nc.scalar.activation(out=scal_sb[:, b * E:(b + 1) * E],
                                     in_=red2[:], func=Act.Sigmoid)

            def emit_mix_ranges(b, kt, ranges, act_experts=ACT_EXPERTS):
                """Mix columns [lo,hi) of sample b's kernel:
                kt[:, j] = sum_e r_be * cv[e][:, j], as an 8-product tree.
                Products: ScalarE (activation scale) for ACT_EXPERTS, VectorE
                tensor_scalar (2x bf16) otherwise. Folds: VectorE tensor_add
                (2x bf16). GpSimd stays out (its FIFO must remain free for
                the routing all_reduce — a fold ahead of it cascades stalls).
                """
                for lo, hi in ranges:
                    sl = slice(lo, hi)
                    for e in range(E):
                        sc = scal_sb[:, b * E + e:b * E + e + 1]
                        if e in act_experts:
                            nc.scalar.activation(out=prod[e][:, sl],
                                                 in_=cv_sb[e][:, sl],
                                                 func=Act.Copy, scale=sc)
                        else:
                            nc.vector.tensor_scalar_mul(out=prod[e][:, sl],
                                                        in0=cv_sb[e][:, sl],
                                                        scalar1=sc)
                    nc.vector.tensor_add(out=prod[4][:, sl], in0=prod[4][:, sl],
                                         in1=prod[5][:, sl])
                    nc.vector.tensor_add(out=prod[6][:, sl], in0=prod[6][:, sl],
                                         in1=prod[7][:, sl])
                    nc.vector.tensor_add(out=prod[0][:, sl], in0=prod[0][:, sl],
                                         in1=prod[1][:, sl])
                    nc.vector.tensor_add(out=prod[2][:, sl], in0=prod[2][:, sl],
                                         in1=prod[3][:, sl])
                    nc.vector.tensor_add(out=prod[4][:, sl], in0=prod[4][:, sl],
                                         in1=prod[6][:, sl])
                    nc.vector.tensor_add(out=prod[0][:, sl], in0=prod[0][:, sl],
                                         in1=prod[2][:, sl])
                    nc.vector.tensor_add(out=kt[:, sl], in0=prod[0][:, sl],
                                         in1=prod[4][:, sl])

            def new_kt():
                return kt_pool.tile([128, KCOLS], bf16, name="kt", tag="kt")

            def emit_mix(b):
                kt = new_kt()
                emit_mix_ranges(b, kt, [(0, KCOLS)])
                return kt

            def emit_conv(b, xp, kt, last=False):
                """Conv for sample b in two tile-groups; drains spread
                mid-sample so PSUM banks recycle before the next sample."""
                x3 = [xp[c].rearrange("p (r q) -> p r q", q=WP)
                      for c in range(CHUNKS)]
                def emit_drain(n, cp, eng_v):
                    o = out_pool.tile([128, NFREE], f32, name="osb", tag="osb")
                    od = out_d[b, :, n * ROWS_PER_TILE:(n + 1) * ROWS_PER_TILE, :]
                    if eng_v == 2:
                        # half-tiles on both engines in parallel
                        hh = NFREE // 2
                        nc.vector.tensor_copy(out=o[:, :hh], in_=cp[:, :hh])
                        nc.scalar.activation(out=o[:, hh:], in_=cp[:, hh:],
                                             func=Act.Copy)
                    elif eng_v:
                        nc.vector.tensor_copy(out=o[:], in_=cp[:])
                    else:
                        nc.scalar.activation(out=o[:], in_=cp[:], func=Act.Copy)
                    nc.sync.dma_start(out=od, in_=o[:])

                for gi, (nlo, nhi) in enumerate(TILE_GROUPS):
                    cps = {n: cps_pool.tile([128, NFREE], f32, name=f"cps{n}",
                                            tag=f"cps{n}")
                           for n in range(nlo, nhi)}
                    if last and gi == 1:
                        # epilogue: tile-major so each tile drains (on both
                        # engines) while the next tile's matmuls still run
                        for n in range(nlo, nhi):
                            for c in range(CHUNKS):
                                for s in range(NSH):
                                    dh, dw = s // KK, s % KK
                                    lhsT = kt[:, c * NSH * 128 + s * 128:
                                              c * NSH * 128 + (s + 1) * 128]
                                    rhs = x3[c][:, n * ROWS_PER_TILE + dh:
                                                n * ROWS_PER_TILE + dh + ROWS_PER_TILE,
                                                dw:dw + W]
                                    nc.tensor.matmul(
                                        cps[n][:], lhsT, rhs,
                                        start=(c == 0 and s == 0),
                                        stop=(c == CHUNKS - 1 and s == NSH - 1))
                            emit_drain(n, cps[n][:], 2)
                        continue
                    for c in range(CHUNKS):
                        for s in range(NSH):
                            dh, dw = s // KK, s % KK
                            lhsT = kt[:, c * NSH * 128 + s * 128:
                                      c * NSH * 128 + (s + 1) * 128]
                            first = (c == 0 and s == 0)
                            last_mm = (c == CHUNKS - 1 and s == NSH - 1)
                            for n in range(nlo, nhi):
                                rhs = x3[c][:, n * ROWS_PER_TILE + dh:
                                            n * ROWS_PER_TILE + dh + ROWS_PER_TILE,
                                            dw:dw + W]
                                nc.tensor.matmul(cps[n][:], lhsT, rhs,
                                                 start=first, stop=last_mm)
                    for i, n in enumerate(range(nlo, nhi)):
                        emit_drain(n, cps[n][:], (gi * 4 + i + 1) % 2)

            # ---- software-pipelined emission ------------------------------
            # DMA ring FIFO order makes the prologue gate xp(0)+bank(c0)
            # (~4.1 MB); sample 0's mix is emitted in 384-col splits so conv
            # matmuls start as soon as that gate clears. Steady state: mix of
            # sample b+1 is emitted before conv(b) so it overlaps on the
            # side engines; input DMAs run 3 samples ahead.
            S = SAMPLES_PER_CORE

            # PE warm-up: ~70 dummy matmuls on a memset scratch keep the PE
            # HAM busy from ~8us so the real conv stream starts at 2.4 GHz.
            # The same scratch preloads the Copy+Sigmoid activation tables
            # (1.3us each) off the routing critical path.
            warm_sb = res_pool.tile([128, 576], bf16, name="warm", tag="warm")
            nc.gpsimd.memset(warm_sb[:], 0)
            wtab = small_pool.tile([128, 1], f32, name="wtab", tag="wtab")
            nc.scalar.activation(out=wtab[:], in_=warm_sb[:, 0:1], func=Act.Copy)
            nc.scalar.activation(out=wtab[:], in_=warm_sb[:, 0:1], func=Act.Sigmoid)
            warm_ps = cps_pool.tile([128, NFREE], f32, name="warmps", tag="warmps")
            for _ in range(48):
                nc.tensor.matmul(warm_ps[:], warm_sb[:, :128], warm_sb[:, 128:],
                                 start=True, stop=True)

            # prologue DMA issue order = per-engine FIFO priority. Bank
            # chunks go in column-splits sized so each mix range's inputs
            # land just before the conv stream consumes its output.
            loads = {0: emit_load(0)}
            hc = NSH * 128  # 1152
            emit_bank_chunk(0, splits=[(0, 576), (576, hc)])
            emit_routing(0, loads[0][1])
            kt0 = new_kt()
            emit_mix_ranges(0, kt0, [(0, 576), (576, hc)])
            emit_bank_chunk(1, splits=[(0, 576), (576, hc)])
            emit_mix_ranges(0, kt0, [(hc, hc + 576), (hc + 576, KCOLS)])
            xp1 = emit_load_dma(1)
            loads[1] = (xp1, emit_load_gap(xp1))
            emit_routing(1, loads[1][1])
            kt1 = new_kt()
            emit_mix_ranges(1, kt1, [(0, 576), (576, hc), (hc, hc + 576),
                                     (hc + 576, KCOLS)])
            loads[2] = emit_load(2)
            emit_routing(2, loads[2][1])
            kts = {0: kt0, 1: kt1}
            emit_conv(0, loads.pop(0)[0], kts.pop(0))
            # steady state: mix(b+1) consumes scal(b+1) written a full
            # iteration ago, so no mix op waits on a same-iteration routing
            # chain; routing(b+2) has the whole conv(b) window to trickle
            # through ScalarE/VectorE/GpSimd.
            for b in range(1, S):
                if b + 1 < S:
                    kts[b + 1] = emit_mix(b + 1)
                if b + 2 < S:
                    loads[b + 2] = emit_load(b + 2)
                    emit_routing(b + 2, loads[b + 2][1])
                emit_conv(b, loads.pop(b)[0], kts.pop(b), last=(b == S - 1))

    nc.compile()
    return nc


def _prep_core_inputs(x, convs, W_att, b_att):
    """Host-side shard/layout prep. Returns list of 8 per-core input dicts."""
    import ml_dtypes
    f32 = np.float32
    bf16 = ml_dtypes.bfloat16
    # padded input, cin split into 2 chunks of 128
    xpad = np.zeros((B, CHUNKS, 128, HP, WP), dtype=bf16)
    xpad[:, :, :, 1:H + 1, 1:W + 1] = np.ascontiguousarray(
        x, dtype=f32).reshape(B, CHUNKS, 128, H, W).astype(bf16)
    xpad = xpad.reshape(B, CHUNKS, 128, PHW)

    # convsT[half][c, cin, e*1152 + s*128 + m] = convs[e, half*128+m, c*128+cin, kh, kw]
    cv = np.ascontiguousarray(convs, dtype=f32).reshape(E, 2, MHALF, CHUNKS, 128, NSH)
    convsT_halves = [
        np.ascontiguousarray(cv[:, h].transpose(2, 3, 0, 4, 1).reshape(
            CHUNKS, 128, E * NSH * 128).astype(bf16))
        for h in range(2)
    ]

    watt = np.ascontiguousarray(
        (np.asarray(W_att, dtype=f32).T / f32(H * W)).reshape(CHUNKS, 128, E))
    battb = np.ascontiguousarray(
        np.broadcast_to(np.asarray(b_att, dtype=f32), (128, E)))

    in_maps = []
    for k in range(NCORES):
        pair, half = k // 2, k % 2
        sl = slice(pair * SAMPLES_PER_CORE, (pair + 1) * SAMPLES_PER_CORE)
        in_maps.append({
            "xpad": np.ascontiguousarray(xpad[sl]),
            "convsT": convsT_halves[half],
            "watt": watt,
            "battb": battb,
        })
    return in_maps


def _assemble_output(results):
    out = np.empty((B, COUT, H, W), dtype=np.float32)
    for k in range(NCORES):
        pair, half = k // 2, k % 2
        sl = slice(pair * SAMPLES_PER_CORE, (pair + 1) * SAMPLES_PER_CORE)
        out[sl, half * MHALF:(half + 1) * MHALF] = results[k]["out"]
    return out


def kernel(x, convs, W_att, b_att):
    from concourse.bass_utils import run_bass_kernel_spmd

    if "nc" not in _cached:
        _cached["nc"] = _build_program()
    in_maps = _prep_core_inputs(x, convs, W_att, b_att)
    res = run_bass_kernel_spmd(_cached["nc"], in_maps, core_ids=list(range(NCORES)))
    return _assemble_output(res.results)
